# revision 1
# baseline (speedup 1.0000x reference)
"""AdaptiveEmbedding T2I sims kernel for 8 TRN2 NeuronCores.

Strategy: shard the caption batch (48 -> 6 per core). Each core holds the
full image tensor, computes BN stats, FiLM params for its 6 captions,
the fovea-softmax weighted pooling, and a [48, 6] slice of the sims
matrix. Host assembles the 8 column slices.

Math notes (all exact, no approximations):
- BatchNorm is a per-channel affine xn = rho*(x - mu). We never
  materialize xn: the affine folds into the exp scale (exp(sg*rho*(x-mm)))
  and into the final FiLM affine u = (g*rho)*E_w[x] + (beta - g*rho*mu).
- softmax max-shift: max_r(a*x_r) = a*xmax if a>0 else a*xmin, so the
  exact per-(i,c,d) shift comes from per-(i,d) min/max reductions and the
  sign of the per-(c,d) FiLM gain. The shift cancels in E_w[x] = sum(e*x)/sum(e).
- beta is constant over regions r, so it drops out of the softmax and
  re-enters linearly: img_vec = (g*E_w[xn] + beta)/R; the 1/R and the
  l2-eps are negligible/cancel in the cosine sim.
"""

import numpy as np
from contextlib import ExitStack

B, T, D, R = 48, 50, 1024, 36
NCORES = 8
CPC = B // NCORES  # captions per core
SMOOTH = 10.0
CLAMP = 80.0
BN_EPS = 1e-5
L2_EPS = 1e-8
P = 128
NBLK = D // P          # 8 d-blocks
NIR = B * R            # 1728 (i*r) rows
NT = (NIR + P - 1) // P  # 14 row-tiles (13 full + 64)

_CACHE = {}
import os as _os
STAGE = int(_os.environ.get("KSTAGE", "9"))

# dtype switches (f32 first; bf16 for the bulk tensors is the perf lever)
XALL_BF16 = True
EP_BF16 = True


def _build_nc():
    import concourse.bass as bass
    import concourse.tile as tile
    from concourse import bacc, mybir
    from concourse.masks import make_identity

    FP = mybir.dt.float32
    BF = mybir.dt.bfloat16
    XDT = BF if XALL_BF16 else FP
    EDT = BF if EP_BF16 else FP
    Alu = mybir.AluOpType
    Act = mybir.ActivationFunctionType

    nc = bacc.Bacc("TRN2", target_bir_lowering=False, debug=False,
                   num_devices=NCORES)

    imgbf = nc.dram_tensor("imgbf", (NIR, D), BF, kind="ExternalInput").ap()
    cap = nc.dram_tensor("cap", (CPC, T, D), FP, kind="ExternalInput").ap()
    maskT_d = nc.dram_tensor("maskT", (T, CPC), FP, kind="ExternalInput").ap()
    wgT_d = nc.dram_tensor("wgT", (D, D), FP, kind="ExternalInput").ap()
    wbT_d = nc.dram_tensor("wbT", (D, D), FP, kind="ExternalInput").ap()
    bg1T_d = nc.dram_tensor("bg1T", (P, NBLK), FP, kind="ExternalInput").ap()
    bbT_d = nc.dram_tensor("bbT", (P, NBLK), FP, kind="ExternalInput").ap()
    out_d = nc.dram_tensor("out", (CPC, B), FP, kind="ExternalOutput").ap()

    with tile.TileContext(nc) as tc, ExitStack() as ctx:
        def _body():
            consts = ctx.enter_context(tc.tile_pool(name="consts", bufs=1))
            ident = consts.tile([P, P], FP, tag="ident")
            make_identity(nc, ident[:])
            ones1 = consts.tile([P, 1], FP, tag="ones1")
            nc.vector.memset(ones1[:], 1.0)

            xall_pool = ctx.enter_context(tc.tile_pool(name="xall", bufs=1))
            xall = [xall_pool.tile([P, B, R], XDT, tag=f"xall{b}", name=f"xall{b}")
                    for b in range(NBLK)]

            smalls = ctx.enter_context(tc.tile_pool(name="smalls", bufs=1))
            tp_psum = ctx.enter_context(tc.tile_pool(name="tp_ps", bufs=2,
                                                     space="PSUM"))

            out_sb_dbg = smalls.tile([CPC, B], FP, tag="out_dbg")
            nc.vector.memset(out_sb_dbg[:], 0.0)

            # ========== Stage C first: caption pooling + capT + norms ==========
            maskT = smalls.tile([T, CPC], FP, tag="maskT")
            nc.sync.dma_start(out=maskT[:], in_=maskT_d[:, :])
            cap_pool = ctx.enter_context(tc.tile_pool(name="cap", bufs=2))
            cap_sb = smalls.tile([CPC, D], FP, tag="cap_sb")
            with tc.tile_pool(name="cap_ps", bufs=2, space="PSUM") as cap_ps_pool:
                for c in range(CPC):
                    ct = cap_pool.tile([T, D], FP, tag="cap")
                    nc.sync.dma_start(out=ct[:], in_=cap[c, :, :])
                    pp = cap_ps_pool.tile([1, D], FP, tag="pp", name="pp")
                    for j in range(2):
                        nc.tensor.matmul(pp[:, 512 * j:512 * (j + 1)],
                                         maskT[:, c:c + 1],
                                         ct[:, 512 * j:512 * (j + 1)],
                                         start=True, stop=True,
                                         skip_group_check=True)
                    prow = cap_pool.tile([1, D], FP, tag="prow", name="prow",
                                         bufs=2)
                    nc.scalar.copy(prow[:], pp[:])
                    nc.sync.dma_start(out=cap_sb[c:c + 1, :], in_=prow[:])

            capT = [smalls.tile([P, CPC], FP, tag=f"capT{b}", name=f"capT{b}")
                    for b in range(NBLK)]
            for blk in range(NBLK):
                pst = tp_psum.tile([P, P], FP, tag="tp")
                nc.tensor.transpose(pst[:, 0:CPC], cap_sb[:, P * blk:P * (blk + 1)],
                                    ident[:CPC, :CPC])
                nc.vector.tensor_copy(out=capT[blk][:], in_=pst[:, 0:CPC])

            scr_c = smalls.tile([CPC, D], FP, tag="scr_c")
            n2 = smalls.tile([CPC, 1], FP, tag="n2")
            nc.vector.tensor_tensor(out=scr_c[:], in0=cap_sb[:], in1=cap_sb[:],
                                    op=Alu.mult)
            nc.vector.tensor_reduce(out=n2[:], in_=scr_c[:],
                                    axis=mybir.AxisListType.X, op=Alu.add)
            nrm = smalls.tile([CPC, 1], FP, tag="nrm")
            nc.scalar.activation(nrm[:], n2[:], Act.Sqrt)
            nrm_e = smalls.tile([CPC, 1], FP, tag="nrm_e")
            nc.vector.tensor_scalar(out=nrm_e[:], in0=nrm[:], scalar1=L2_EPS,
                                    scalar2=None, op0=Alu.add)
            rn = smalls.tile([CPC, 1], FP, tag="rn")
            nc.vector.reciprocal(rn[:], nrm_e[:])

            # ========== Stage A: DMA-transpose img (bf16) + DVE stats ==========
            for blk in range(NBLK):
                nc.sync.dma_start_transpose(
                    out=xall[blk][:].rearrange("p i r -> p (i r)"),
                    in_=imgbf[:, P * blk:P * (blk + 1)])

            muT = smalls.tile([P, NBLK], FP, tag="muT")
            m2T = smalls.tile([P, NBLK], FP, tag="m2T")
            sq_pool = ctx.enter_context(tc.tile_pool(name="sq", bufs=2))
            inv_n = 1.0 / float(NIR)
            for blk in range(NBLK):
                sums = smalls.tile([P, 1], FP, tag="sums")
                nc.vector.tensor_reduce(
                    out=sums[:], in_=xall[blk][:].rearrange("p i r -> p (i r)"),
                    axis=mybir.AxisListType.X, op=Alu.add)
                nc.vector.tensor_scalar(out=muT[:, blk:blk + 1], in0=sums[:],
                                        scalar1=inv_n, scalar2=None, op0=Alu.mult)
                x2t = sq_pool.tile([P, B * R], FP, tag="x2t")
                nc.scalar.square(x2t[:], xall[blk][:].rearrange("p i r -> p (i r)"))
                sq = smalls.tile([P, 1], FP, tag="sqs")
                nc.vector.tensor_reduce(out=sq[:], in_=x2t[:],
                                        axis=mybir.AxisListType.X, op=Alu.add)
                nc.vector.tensor_scalar(out=m2T[:, blk:blk + 1], in0=sq[:],
                                        scalar1=inv_n, scalar2=None, op0=Alu.mult)
            musqT = smalls.tile([P, NBLK], FP, tag="musqT")
            nc.scalar.square(musqT[:], muT[:])
            varT = smalls.tile([P, NBLK], FP, tag="varT")
            nc.vector.tensor_tensor(out=varT[:], in0=m2T[:], in1=musqT[:],
                                    op=Alu.subtract)
            varTe = smalls.tile([P, NBLK], FP, tag="varTe")
            nc.vector.tensor_scalar(out=varTe[:], in0=varT[:], scalar1=BN_EPS,
                                    scalar2=None, op0=Alu.add)
            stdT = smalls.tile([P, NBLK], FP, tag="stdT")
            nc.scalar.activation(stdT[:], varTe[:], Act.Sqrt)
            rhoT = smalls.tile([P, NBLK], FP, tag="rhoT")
            nc.vector.reciprocal(rhoT[:], stdT[:])

            # ========== Stage D: FiLM params, capT-stationary ==========
            bg1T = smalls.tile([P, NBLK], FP, tag="bg1T")
            nc.sync.dma_start(out=bg1T[:], in_=bg1T_d[:, :])
            bbT = smalls.tile([P, NBLK], FP, tag="bbT")
            nc.sync.dma_start(out=bbT[:], in_=bbT_d[:, :])

            aT = [smalls.tile([P, CPC], FP, tag=f"aT{b}", name=f"aT{b}")
                  for b in range(NBLK)]
            b2T = [smalls.tile([P, CPC], FP, tag=f"b2T{b}", name=f"b2T{b}")
                   for b in range(NBLK)]
            scaleT = [smalls.tile([P, CPC], FP, tag=f"scT{b}", name=f"scT{b}")
                      for b in range(NBLK)]
            thrT = [smalls.tile([P, CPC], FP, tag=f"thrT{b}", name=f"thrT{b}")
                    for b in range(NBLK)]
            nthrT = [smalls.tile([P, CPC], FP, tag=f"nthrT{b}", name=f"nthrT{b}")
                     for b in range(NBLK)]

            w_pool = ctx.enter_context(tc.tile_pool(name="w", bufs=3))
            gcd_pool = ctx.enter_context(tc.tile_pool(name="gcd", bufs=2))
            with tc.tile_pool(name="gb_ps", bufs=4, space="PSUM") as gb_ps_pool:
                for which, wd in (("g", wgT_d), ("b", wbT_d)):
                    for half in range(2):
                        ps = gb_ps_pool.tile([CPC, 512], FP, tag="gcd",
                                             name="gcd_ps")
                        for kb in range(NBLK):
                            w = w_pool.tile([P, D // 2], FP, tag="w", name="w")
                            nc.sync.dma_start(
                                out=w[:], in_=wd[P * kb:P * (kb + 1),
                                                 512 * half:512 * (half + 1)])
                            nc.tensor.matmul(ps[:], capT[kb][:], w[:],
                                             start=(kb == 0),
                                             stop=(kb == NBLK - 1),
                                             skip_group_check=True)
                        gsb = gcd_pool.tile([CPC, 512], FP, tag="gsb", name="gsb")
                        nc.scalar.copy(gsb[:], ps[:])
                        for j in range(4):
                            db = half * 4 + j
                            pst = tp_psum.tile([P, P], FP, tag="tp")
                            nc.tensor.transpose(pst[:, 0:CPC],
                                                gsb[:, P * j:P * (j + 1)],
                                                ident[:CPC, :CPC])
                            if which == "g":
                                gp1 = smalls.tile([P, CPC], FP, tag=f"gp1_{db}",
                                                  name=f"gp1_{db}")
                                nc.vector.tensor_scalar(out=gp1[:],
                                                        in0=pst[:, 0:CPC],
                                                        scalar1=bg1T[:, db:db + 1],
                                                        scalar2=None, op0=Alu.add)
                                nc.vector.tensor_scalar(out=aT[db][:], in0=gp1[:],
                                                        scalar1=rhoT[:, db:db + 1],
                                                        scalar2=None, op0=Alu.mult)
                                nc.vector.tensor_scalar(out=scaleT[db][:],
                                                        in0=aT[db][:],
                                                        scalar1=SMOOTH,
                                                        scalar2=None, op0=Alu.mult)
                                negsc = smalls.tile([P, CPC], FP, tag="negsc")
                                nc.vector.tensor_scalar(out=negsc[:],
                                                        in0=scaleT[db][:],
                                                        scalar1=-1.0, scalar2=None,
                                                        op0=Alu.mult)
                                absT = smalls.tile([P, CPC], FP, tag="absT")
                                nc.vector.tensor_tensor(out=absT[:],
                                                        in0=scaleT[db][:],
                                                        in1=negsc[:], op=Alu.max)
                                rabs = smalls.tile([P, CPC], FP, tag="rabs")
                                nc.vector.reciprocal(rabs[:], absT[:])
                                nc.vector.tensor_scalar(out=thrT[db][:],
                                                        in0=rabs[:],
                                                        scalar1=CLAMP,
                                                        scalar2=None, op0=Alu.mult)
                                nc.vector.tensor_scalar(out=nthrT[db][:],
                                                        in0=thrT[db][:],
                                                        scalar1=-1.0, scalar2=None,
                                                        op0=Alu.mult)
                            else:
                                betat = smalls.tile([P, CPC], FP, tag=f"bet_{db}",
                                                    name=f"bet_{db}")
                                nc.vector.tensor_scalar(out=betat[:],
                                                        in0=pst[:, 0:CPC],
                                                        scalar1=bbT[:, db:db + 1],
                                                        scalar2=None, op0=Alu.add)
                                amu = smalls.tile([P, CPC], FP, tag="amu")
                                nc.vector.tensor_scalar(out=amu[:], in0=aT[db][:],
                                                        scalar1=muT[:, db:db + 1],
                                                        scalar2=None, op0=Alu.mult)
                                nc.vector.tensor_tensor(out=b2T[db][:],
                                                        in0=betat[:], in1=amu[:],
                                                        op=Alu.subtract)

            if STAGE < 5:
                nc.sync.dma_start(out=out_d[:, :], in_=out_sb_dbg[:])
                return

            # ========== Stage F: main loop ==========
            big_pool = ctx.enter_context(tc.tile_pool(name="big", bufs=3))
            sc_pool = ctx.enter_context(tc.tile_pool(name="sc", bufs=6))
            out_sb = smalls.tile([CPC, B], FP, tag="out_sb")
            dots_sb = smalls.tile([CPC, B], FP, tag="dots_sb")
            usq_sb = smalls.tile([CPC, B], FP, tag="usq_sb")
            with tc.tile_pool(name="dot_ps", bufs=2, space="PSUM") as dot_ps_pool:
                for c in range(CPC):
                    ps_dot = dot_ps_pool.tile([1, B], FP, tag="dot")
                    ps_usq = dot_ps_pool.tile([1, B], FP, tag="usq")
                    for blk in range(NBLK):
                        d1 = big_pool.tile([P, B, R], XDT, tag="d1", bufs=2)
                        nc.vector.tensor_scalar(out=d1[:], in0=xall[blk][:],
                                                scalar1=thrT[blk][:, c:c + 1],
                                                scalar2=nthrT[blk][:, c:c + 1],
                                                op0=Alu.min, op1=Alu.max)
                        e = big_pool.tile([P, B, R], EDT, tag="e", bufs=4)
                        nc.scalar.activation(e[:], d1[:], Act.Exp,
                                             scale=scaleT[blk][:, c:c + 1])
                        p = big_pool.tile([P, B, R], EDT, tag="p", bufs=4)
                        nc.gpsimd.tensor_tensor(out=p[:], in0=e[:],
                                                in1=xall[blk][:], op=Alu.mult)
                        s = sc_pool.tile([P, B], FP, tag="s")
                        nc.vector.tensor_reduce(out=s[:], in_=e[:],
                                                axis=mybir.AxisListType.X,
                                                op=Alu.add)
                        w = sc_pool.tile([P, B], FP, tag="wr")
                        nc.vector.tensor_reduce(out=w[:], in_=p[:],
                                                axis=mybir.AxisListType.X,
                                                op=Alu.add)
                        rs = sc_pool.tile([P, B], FP, tag="rs")
                        nc.vector.reciprocal(rs[:], s[:])
                        wa = sc_pool.tile([P, B], FP, tag="wa")
                        nc.vector.scalar_tensor_tensor(
                            out=wa[:], in0=w[:], scalar=aT[blk][:, c:c + 1],
                            in1=rs[:], op0=Alu.mult, op1=Alu.mult)
                        u = sc_pool.tile([P, B], FP, tag="u")
                        nc.vector.tensor_scalar(out=u[:], in0=wa[:],
                                                scalar1=b2T[blk][:, c:c + 1],
                                                scalar2=None, op0=Alu.add)
                        uu = sc_pool.tile([P, B], FP, tag="uu")
                        nc.scalar.square(uu[:], u[:])
                        nc.tensor.matmul(ps_dot[:], capT[blk][:, c:c + 1], u[:],
                                         start=(blk == 0), stop=(blk == NBLK - 1),
                                         skip_group_check=True)
                        nc.tensor.matmul(ps_usq[:], ones1[:], uu[:],
                                         start=(blk == 0), stop=(blk == NBLK - 1),
                                         skip_group_check=True)
                    drow = sc_pool.tile([1, B], FP, tag="drow")
                    nc.scalar.copy(drow[:], ps_dot[:])
                    urow = sc_pool.tile([1, B], FP, tag="urow")
                    nc.scalar.copy(urow[:], ps_usq[:])
                    nc.sync.dma_start(out=dots_sb[c:c + 1, :], in_=drow[:])
                    nc.sync.dma_start(out=usq_sb[c:c + 1, :], in_=urow[:])
            sq = smalls.tile([CPC, B], FP, tag="sqf")
            nc.scalar.activation(sq[:], usq_sb[:], Act.Sqrt)
            ru = smalls.tile([CPC, B], FP, tag="ruf")
            nc.vector.reciprocal(ru[:], sq[:])
            t1 = smalls.tile([CPC, B], FP, tag="t1f")
            nc.vector.tensor_tensor(out=t1[:], in0=dots_sb[:], in1=ru[:],
                                    op=Alu.mult)
            nc.vector.tensor_scalar(out=out_sb[:], in0=t1[:],
                                    scalar1=rn[:, 0:1], scalar2=None, op0=Alu.mult)
            nc.sync.dma_start(out=out_d[:, :], in_=out_sb[:])

        _body()
    nc.compile()
    return nc


def _get_nc():
    if "nc" not in _CACHE:
        _CACHE["nc"] = _build_nc()
    return _CACHE["nc"]


def kernel(img_embed, cap_embed, lens, W_gamma, b_gamma, W_beta, b_beta,
           _want_trace=False):
    from concourse.bass_utils import run_bass_kernel_spmd

    nc = _get_nc()

    img_embed = np.asarray(img_embed, np.float32)
    cap_embed = np.asarray(cap_embed, np.float32)
    lens_np = np.asarray(lens)
    W_gamma = np.asarray(W_gamma, np.float32)
    W_beta = np.asarray(W_beta, np.float32)
    b_gamma = np.asarray(b_gamma, np.float32)
    b_beta = np.asarray(b_beta, np.float32)

    import ml_dtypes
    img_bf = np.ascontiguousarray(
        img_embed.reshape(NIR, D).astype(ml_dtypes.bfloat16))
    wgT = np.ascontiguousarray(W_gamma.T)
    wbT = np.ascontiguousarray(W_beta.T)
    bg1T = np.ascontiguousarray((1.0 + b_gamma).reshape(NBLK, P).T)
    bbT = np.ascontiguousarray(b_beta.reshape(NBLK, P).T)

    lens_f = lens_np.astype(np.float64)
    mask = (np.arange(T)[None, :] < lens_np[:, None]).astype(np.float64)
    mask = (mask / lens_f[:, None]).astype(np.float32)  # (B, T)

    in_maps = []
    for k in range(NCORES):
        sl = slice(k * CPC, (k + 1) * CPC)
        in_maps.append({
            "imgbf": img_bf,
            "cap": np.ascontiguousarray(cap_embed[sl]),
            "maskT": np.ascontiguousarray(mask[sl].T),
            "wgT": wgT,
            "wbT": wbT,
            "bg1T": bg1T,
            "bbT": bbT,
        })

    kw = {}
    if _want_trace:
        import os as _os2, shutil as _sh
        _sh.rmtree("/tmp/ktrace", ignore_errors=True)
        _os2.makedirs("/tmp/ktrace", exist_ok=True)
        kw = {"tmpdir": "/tmp/ktrace"}
    res = run_bass_kernel_spmd(nc, in_maps, core_ids=list(range(NCORES)),
                               trace=_want_trace, **kw)
    outs = [np.asarray(r["out"]) for r in res.results]
    sims = np.concatenate([o.T for o in outs], axis=1).astype(np.float32)
    if _want_trace:
        return sims, res
    return sims

